# revision 1
# baseline (speedup 1.0000x reference)
"""Trainium2 Bass kernel for nn_Attention (Bahdanau-style attention decode step).

Reference computation (per batch b):
    h_proj  = hidden[b] @ W_h                      # [1, H]
    e_proj  = enc[b] @ W_e                         # [S, H]
    energy  = tanh(e_proj + h_proj + bias)         # [S, H]
    attn    = energy @ v                           # [S]
    w       = softmax(attn)                        # [S]
    context = w @ enc[b]                           # [E]

Sharding: data-parallel over batch on 8 cores (4 batches/core), no collectives.

Per-core kernel strategy (all matmuls in float32r = full-speed ~tf32):
  - enc tiles DMA'd in natural [s=128p, e] layout (contiguous rows).
  - PE transpose (128x128 blocks) produces encT [e=128p, s] for the main matmul.
  - Main matmul emits transposed energy [h=128p, s=512] per h-tile; the
    (h_proj + bias) term is a per-partition scalar there, so it folds into the
    tanh via ACT's bias operand.
  - attention = v.T @ energyT directly on PE (v as 128x1 stationary).
  - exp on ACT with accum_out accumulating the softmax denominator for free.
    Max-subtraction is skipped: |logits| <= sum|v| ~ 16, safe in fp32.
  - exp rows transposed back to [s=128p] columns on PE; context accumulates
    on PE against the natural-layout enc tiles already in SBUF.
  - The softmax/context chain for chunk c is software-pipelined into chunk
    c+1's instruction stream so the PE never head-blocks on it.
"""

import numpy as np

HIDDEN = 1024
ENC = 1024
BATCH = 32
SEQ = 2048
NCORES = 8
B_LOC = BATCH // NCORES  # 4

S_CHUNK = 512
N_CHUNK = SEQ // S_CHUNK  # 4
SUBS = S_CHUNK // 128  # 4
ET = ENC // 128  # 8 e-tiles
HT = HIDDEN // 128  # 8 h-tiles

_CACHED_NC = None


def build_bass(b_loc=B_LOC, seq=SEQ, repeat=1, ablate=(), tune=None):
    ablate = set(ablate)
    T = {
        "drain_a": 2,      # transpose group index after which stage A drains
        "drain_b": -1,     # h-tile index for stage B drain (-1 = before h-loop)
        "nat_bufs": 12,
        "encT_bufs": 16,
        "energyT_bufs": 6,
        "tp_bufs": 2,
        "attn_bufs": 2,
        "dve_ctx": False,  # context via DVE ttr on encT instead of PE matmuls
    }
    T.update(tune or {})
    import concourse.mybir as mybir
    import concourse.tile as tile
    from concourse import bacc
    from concourse.bass import ts
    from concourse.masks import make_identity

    n_chunk = seq // S_CHUNK

    nc = bacc.Bacc()
    R = mybir.dt.float32r
    F = mybir.dt.float32
    AF = mybir.ActivationFunctionType

    hidden = nc.dram_tensor("hidden", [b_loc, HIDDEN], R, kind="ExternalInput")
    enc = nc.dram_tensor("enc", [b_loc, seq, ENC], R, kind="ExternalInput")
    attn_w = nc.dram_tensor("attn_w", [HIDDEN + ENC, HIDDEN], R, kind="ExternalInput")
    attn_b = nc.dram_tensor("attn_b", [HIDDEN], F, kind="ExternalInput")
    v_w = nc.dram_tensor("v_w", [HIDDEN], R, kind="ExternalInput")
    out = nc.dram_tensor("out", [b_loc, ENC], F, kind="ExternalOutput")
    scratch_rz = nc.dram_tensor("scratch_rz", [b_loc, 1], F)
    scratch_exp = nc.dram_tensor("scratch_exp", [seq // S_CHUNK, S_CHUNK], F)

    with tile.TileContext(nc) as tc:
        with (
            tc.tile_pool(name="weights", bufs=1) as w_pool,
            tc.tile_pool(name="consts", bufs=1) as const_pool,
            tc.tile_pool(name="nat", bufs=T["nat_bufs"]) as nat_pool,
            tc.tile_pool(name="encT", bufs=T["encT_bufs"]) as encT_pool,
            tc.tile_pool(name="energyT", bufs=T["energyT_bufs"]) as energyT_pool,
            tc.tile_pool(name="small", bufs=8) as small_pool,
            tc.tile_pool(name="mid", bufs=3) as mid_pool,
            tc.tile_pool(name="ps_tp", bufs=T["tp_bufs"], space="PSUM") as tp_pool,
            tc.tile_pool(name="ps_main", bufs=2, space="PSUM") as main_pool,
            tc.tile_pool(name="ps_attn", bufs=T["attn_bufs"], space="PSUM") as attn_pool,
            tc.tile_pool(name="ps_ctx", bufs=1, space="PSUM") as ctx_pool,
        ):
            # ---- constants / weights ----
            ident_f = const_pool.tile([128, 128], F, tag="ident_f")
            make_identity(nc, ident_f[:])
            ident = const_pool.tile([128, 128], R)
            nc.vector.tensor_copy(ident[:], ident_f[:])

            w_h = w_pool.tile([128, ET, HIDDEN], R, tag="w_h")
            w_e = w_pool.tile([128, ET, HIDDEN], R, tag="w_e")
            # attn_w rows [0:1024] are W_h, [1024:2048] are W_e
            nc.sync.dma_start(
                w_h[:], attn_w[0:HIDDEN, :].rearrange("(j p) h -> p j h", p=128)
            )
            nc.sync.dma_start(
                w_e[:],
                attn_w[HIDDEN : HIDDEN + ENC, :].rearrange("(j p) h -> p j h", p=128),
            )

            # ones row for rank-1 partition-broadcast matmuls
            ones_f = const_pool.tile([1, 128], F, tag="ones_f")
            nc.vector.memset(ones_f[:], 1.0)
            ones_r = const_pool.tile([1, 128], R, tag="ones_r")
            nc.vector.tensor_copy(ones_r[:], ones_f[:])

            # v as [h=128p, j] columns
            vT2 = const_pool.tile([128, HT, 2], R, tag="vT2")
            for k in range(2):
                nc.gpsimd.dma_start(
                    out=vT2[:, :, k], in_=v_w[:].rearrange("(j p) -> p j", p=128)
                )

            # ---- preamble: hb[b, h] = hidden[b] @ W_h + attn_b, laid out as
            # hbT [h=128p, j, b] per-partition bias columns ----
            hidden_nat = const_pool.tile([b_loc, HIDDEN], R, tag="hidden_nat")
            nc.sync.dma_start(hidden_nat[:], hidden[:, :])

            hiddenT = const_pool.tile([128, ET, b_loc], R, tag="hiddenT")
            for j in range(ET):
                pt = tp_pool.tile([128, 512], R, tag="tp")
                nc.tensor.matmul(
                    pt[:, 0:b_loc],
                    hidden_nat[0:b_loc, ts(j, 128)],
                    ident[0:b_loc, 0:b_loc],
                    is_transpose=True,
                    start=True,
                    stop=True,
                )
                nc.vector.tensor_copy(hiddenT[:, j, :], pt[:, 0:b_loc])

            attnb_sb = const_pool.tile([b_loc, HIDDEN], F, tag="attnb")
            nc.gpsimd.dma_start(
                out=attnb_sb[:], in_=attn_b[:].partition_broadcast(b_loc)
            )
            hb_ps = ctx_pool.tile([b_loc, HIDDEN], F, tag="ctx")
            for n in range(2):
                for j in range(ET):
                    nc.tensor.matmul(
                        hb_ps[:, ts(n, 512)],
                        hiddenT[:, j, :],
                        w_h[:, j, ts(n, 512)],
                        start=(j == 0),
                        stop=(j == ET - 1),
                    )
            hb_nat = const_pool.tile([b_loc, HIDDEN], R, tag="hb_nat")
            nc.vector.tensor_add(hb_nat[:], hb_ps[:], attnb_sb[:])

            hbT = const_pool.tile([128, HT, b_loc], R, tag="hbT")
            for j in range(HT):
                pt = tp_pool.tile([128, 512], R, tag="tp")
                nc.tensor.matmul(
                    pt[:, 0:b_loc],
                    hb_nat[0:b_loc, ts(j, 128)],
                    ident[0:b_loc, 0:b_loc],
                    is_transpose=True,
                    start=True,
                    stop=True,
                )
                nc.vector.tensor_copy(hbT[:, j, :], pt[:, 0:b_loc])

            # ---- main loop ----
            # Cross-chunk software pipeline: the softmax/context path for
            # chunk c is emitted inside chunk c+1's block (after its
            # transposes), so PE never head-blocks on the exp chain; the
            # per-batch finalize is likewise deferred into the next chunk.
            def emit_ctx_exp_dve(c, zparts, attn_ps):
                # exp, then broadcast the exp row across all 128 partitions so
                # DVE can weight encT columns with it
                if "ctx" in ablate or "attn" in ablate:
                    return None
                exp_row = mid_pool.tile([2, S_CHUNK], R, tag="exp_row")
                nc.scalar.activation(
                    exp_row[:],
                    attn_ps[:],
                    AF.Exp,
                    accum_out=zparts[0:2, c : c + 1],
                )
                nc.sync.dma_start(scratch_exp[c : c + 1, :], exp_row[0:1, :].bitcast(F))
                expb = mid_pool.tile([128, S_CHUNK], F, tag="expb", name="expb")
                nc.gpsimd.dma_start(
                    out=expb[:], in_=scratch_exp[c, :].partition_broadcast(128)
                )
                return expb

            def emit_ctx_ttr(c, ctx_col, expb, encTs_c):
                # context partials on DVE: ctx_col[e=128p, j] +=
                #   sum_s encT_j[e, s] * exp[s]
                if "ctx" in ablate or "attn" in ablate:
                    return
                for j in range(ET):
                    scratch = mid_pool.tile([128, S_CHUNK], F, tag="ttr_scr")
                    nc.vector.tensor_tensor_reduce(
                        out=scratch[:],
                        in0=encTs_c[j][:].bitcast(F),
                        in1=expb[:],
                        scale=1.0,
                        scalar=(0.0 if c == 0 else ctx_col[:, j : j + 1]),
                        op0=mybir.AluOpType.mult,
                        op1=mybir.AluOpType.add,
                        accum_out=ctx_col[:, j : j + 1],
                    )

            def emit_finalize_dve(b, ctx_col, zparts):
                if "ctx" in ablate or "attn" in ablate:
                    return
                zsum = small_pool.tile([1, 1], F, tag="zsum")
                nc.vector.tensor_reduce(
                    zsum[:],
                    zparts[0:1, :],
                    mybir.AxisListType.X,
                    mybir.AluOpType.add,
                )
                rz = small_pool.tile([1, 1], F, tag="rz")
                nc.vector.reciprocal(rz[:], zsum[:])
                nc.sync.dma_start(scratch_rz[b : b + 1, :], rz[:])
                rzb = small_pool.tile([128, 1], F, tag="rzb")
                nc.gpsimd.dma_start(
                    out=rzb[:], in_=scratch_rz[b, :].partition_broadcast(128)
                )
                ctx_fin = small_pool.tile([128, ET], F, tag="ctx_fin")
                nc.vector.tensor_scalar_mul(ctx_fin[:], ctx_col[:], rzb[:])
                nc.sync.dma_start(
                    out[b, :].rearrange("(j p) -> p j", p=128), ctx_fin[:]
                )

            def emit_ctx_exp(c, zparts, attn_ps):
                # stage A of the deferred softmax path: exp + transposes + copies
                if "ctx" in ablate or "attn" in ablate:
                    return []
                exp_row = mid_pool.tile([2, S_CHUNK], R, tag="exp_row")
                nc.scalar.activation(
                    exp_row[:],
                    attn_ps[:],
                    AF.Exp,
                    accum_out=zparts[0:2, c : c + 1],
                )
                pts = []
                for t in range(SUBS):
                    pt = tp_pool.tile([128, 2], R, tag="tp")
                    nc.tensor.matmul(
                        pt[:],
                        exp_row[0:2, ts(t, 128)],
                        ident[0:2, 0:2],
                        is_transpose=True,
                        start=True,
                        stop=True,
                    )
                    pts.append(pt)
                ecs = []
                for t in range(SUBS):
                    ec = small_pool.tile([128, 2], R, tag="ec")
                    nc.vector.tensor_copy(ec[:], pts[t][:])
                    ecs.append(ec)
                return ecs

            def emit_ctx_mms_sub(c, ctx_ps, ecs, nats, t):
                if "ctx" in ablate or "attn" in ablate:
                    return
                for n in range(2):
                    nc.tensor.matmul(
                        ctx_ps[:, ts(n, 512)],
                        ecs[t][:, 0:1],
                        nats[t][:, ts(n, 512)],
                        start=(c == 0 and t == 0),
                        stop=(c == n_chunk - 1 and t == SUBS - 1),
                    )

            def emit_ctx_mms(c, ctx_ps, ecs, nats):
                # stage B: context accumulation against the natural enc tiles
                for t in range(SUBS):
                    emit_ctx_mms_sub(c, ctx_ps, ecs, nats, t)

            def emit_finalize(b, ctx_ps, zparts):
                if "ctx" in ablate or "attn" in ablate:
                    return
                zsum = small_pool.tile([1, 1], F, tag="zsum")
                nc.vector.tensor_reduce(
                    zsum[:],
                    zparts[0:1, :],
                    mybir.AxisListType.X,
                    mybir.AluOpType.add,
                )
                rz = small_pool.tile([1, 1], F, tag="rz")
                nc.vector.reciprocal(rz[:], zsum[:])
                ctx_sb = mid_pool.tile([1, ENC], F, tag="ctx_sb")
                nc.vector.tensor_scalar_mul(ctx_sb[:], ctx_ps[:], rz[:])
                nc.sync.dma_start(out[b : b + 1, :], ctx_sb[:])

            def emit_main():
                pending_a = []  # deferred stage-A closures (exp chain)
                pending_b = []  # deferred stage-B closures (ctx MMs, finalize)
                for b in range(b_loc):
                    if T["dve_ctx"]:
                        ctx_ps = None
                        ctx_col = mid_pool.tile(
                            [128, ET], F, tag="ctx_col", name="ctx_col"
                        )
                    else:
                        ctx_ps = ctx_pool.tile([1, ENC], F, tag="ctx", name="ctx_ps")
                        ctx_col = None
                    zparts = small_pool.tile([2, n_chunk], F, tag="zparts", name="zparts")
                    for c in range(n_chunk):
                        # load natural enc tiles [s=128, e=1024]
                        nats = []
                        for t in range(SUBS):
                            nat = nat_pool.tile([128, ENC], R, tag="nat", name="nat")
                            s0 = c * S_CHUNK + t * 128
                            nc.sync.dma_start(nat[:], enc[b, s0 : s0 + 128, :])
                            nats.append(nat)
                        # transpose to encT [e=128p, s=512] per e-tile
                        encTs = []
                        for j in range(ET):
                            eT = encT_pool.tile([128, S_CHUNK], R, tag="encT", name="eT")
                            if "transpose" not in ablate:
                                pt = tp_pool.tile([128, 512], R, tag="tp", name="pt")
                                for t in range(SUBS):
                                    nc.tensor.matmul(
                                        pt[:, ts(t, 128)],
                                        nats[t][:, ts(j, 128)],
                                        ident[:],
                                        is_transpose=True,
                                        start=(t == 0),
                                        stop=(t == SUBS - 1),
                                    )
                                nc.vector.tensor_copy(eT[:], pt[:])
                            encTs.append(eT)
                            # drain deferred work from the previous chunk in
                            # two stages, interleaved with the transpose
                            # groups so PE never waits on the exp chain.
                            if j == T["drain_a"]:
                                for fn in pending_a:
                                    fn()
                                del pending_a[:]
                            if T["drain_b"] == "spread" and j >= SUBS and pending_b:
                                pending_b.pop(0)()
                        if T["drain_b"] == "spread":
                            while pending_b:
                                pending_b.pop(0)()
                        elif T["drain_b"] < 0:
                            for fn in pending_b:
                                fn()
                            del pending_b[:]
                        # main matmul + tanh + attention accumulation
                        attn_ps = attn_pool.tile([2, S_CHUNK], F, tag="attn", name="attn_ps")
                        ets = []
                        for i in range(HT):
                            pm = main_pool.tile([128, S_CHUNK], F, tag="main", name="pm")
                            if "main" not in ablate:
                                for j in range(ET):
                                    nc.tensor.matmul(
                                        pm[:],
                                        w_e[:, j, ts(i, 128)],
                                        encTs[j][:],
                                        start=(j == 0),
                                        stop=(j == ET - 1),
                                    )
                            et = energyT_pool.tile([128, S_CHUNK], R, tag="energyT", name="et")
                            if "tanh" not in ablate and "main" not in ablate:
                                nc.scalar.activation(
                                    et[:], pm[:], AF.Tanh, bias=hbT[:, i, b : b + 1]
                                )
                            ets.append(et)
                            # attention matmul for h-tile i-1: its tanh has had
                            # a full h-tile of main matmuls to complete.
                            if "attn" not in ablate and i > 0:
                                nc.tensor.matmul(
                                    attn_ps[:],
                                    vT2[:, i - 1, :],
                                    ets[i - 1][:],
                                    start=(i - 1 == 0),
                                    stop=False,
                                )
                            if i == T["drain_b"]:
                                for fn in pending_b:
                                    fn()
                                del pending_b[:]
                        if "attn" not in ablate:
                            nc.tensor.matmul(
                                attn_ps[:],
                                vT2[:, HT - 1, :],
                                ets[HT - 1][:],
                                start=False,
                                stop=True,
                            )
                        state = {}

                        def stage_a(c=c, zparts=zparts, attn_ps=attn_ps, state=state):
                            state["ecs"] = emit_ctx_exp(c, zparts, attn_ps)

                        if T["dve_ctx"]:

                            def stage_a_dve(
                                c=c, zparts=zparts, attn_ps=attn_ps, state=state
                            ):
                                state["expb"] = emit_ctx_exp_dve(c, zparts, attn_ps)

                            def stage_b_dve(
                                c=c, ctx_col=ctx_col, encTs_c=encTs, state=state
                            ):
                                emit_ctx_ttr(c, ctx_col, state["expb"], encTs_c)

                            pending_a.append(stage_a_dve)
                            pending_b.append(stage_b_dve)
                        elif T["drain_b"] == "spread":
                            pending_a.append(stage_a)
                            for t in range(SUBS):
                                pending_b.append(
                                    lambda c=c, ctx_ps=ctx_ps, nats=nats,
                                    state=state, t=t: emit_ctx_mms_sub(
                                        c, ctx_ps, state["ecs"], nats, t
                                    )
                                )
                        else:
                            pending_a.append(stage_a)
                            pending_b.append(
                                lambda c=c, ctx_ps=ctx_ps, nats=nats,
                                state=state: emit_ctx_mms(
                                    c, ctx_ps, state["ecs"], nats
                                )
                            )
                    if T["dve_ctx"]:
                        pending_b.append(
                            lambda b=b, ctx_col=ctx_col, zparts=zparts: (
                                emit_finalize_dve(b, ctx_col, zparts)
                            )
                        )
                    else:
                        pending_b.append(
                            lambda b=b, ctx_ps=ctx_ps, zparts=zparts: emit_finalize(
                                b, ctx_ps, zparts
                            )
                        )
                for fn in pending_a:
                    fn()
                for fn in pending_b:
                    fn()

            if repeat > 1:
                with tc.For_i(0, repeat, 1):
                    emit_main()
            else:
                emit_main()

    nc.compile()
    return nc


def kernel_run(hidden, encoder_outputs, attn_w, attn_b, v_w, **spmd_kwargs):
    """Shards over batch across 8 cores, runs the Bass kernel SPMD, gathers
    per-core outputs. Returns (full_output, BassKernelResults)."""
    global _CACHED_NC
    from concourse.bass_utils import run_bass_kernel_spmd

    if _CACHED_NC is None:
        _CACHED_NC = build_bass()
    nc = _CACHED_NC

    hidden = np.asarray(hidden, dtype=np.float32).reshape(BATCH, HIDDEN)
    enc = np.ascontiguousarray(np.asarray(encoder_outputs, dtype=np.float32))
    attn_w = np.ascontiguousarray(np.asarray(attn_w, dtype=np.float32))
    attn_b = np.ascontiguousarray(np.asarray(attn_b, dtype=np.float32))
    v_w = np.ascontiguousarray(np.asarray(v_w, dtype=np.float32))

    in_maps = []
    for c in range(NCORES):
        lo, hi = c * B_LOC, (c + 1) * B_LOC
        in_maps.append(
            {
                "hidden": np.ascontiguousarray(hidden[lo:hi]),
                "enc": np.ascontiguousarray(enc[lo:hi]),
                "attn_w": attn_w,
                "attn_b": attn_b,
                "v_w": v_w,
            }
        )

    res = run_bass_kernel_spmd(
        nc, in_maps, core_ids=list(range(NCORES)), **spmd_kwargs
    )
    outs = [r["out"] for r in res.results]
    full = np.concatenate(outs, axis=0).reshape(BATCH, 1, ENC)
    return full, res


def kernel(hidden, encoder_outputs, attn_w, attn_b, v_w):
    """Full-input entry point: takes the full (unsharded) inputs, returns the
    full [32, 1, 1024] output."""
    full, _ = kernel_run(hidden, encoder_outputs, attn_w, attn_b, v_w)
    return full



# revision 27
# speedup vs baseline: 1.3951x; 1.3951x over previous
"""Trainium2 Bass kernel for nn_Attention (Bahdanau-style attention decode step).

Reference computation (per batch b):
    h_proj  = hidden[b] @ W_h                      # [1, H]
    e_proj  = enc[b] @ W_e                         # [S, H]
    energy  = tanh(e_proj + h_proj + bias)         # [S, H]
    attn    = energy @ v                           # [S]
    w       = softmax(attn)                        # [S]
    context = w @ enc[b]                           # [E]

Sharding: data-parallel over batch on 8 cores (4 batches/core), no collectives.

Per-core kernel strategy (v3 — fp8 DoubleRow main matmul, bf16 nat):
  - enc loaded one cast-DMA per 512-seq chunk (SWDGE fp32->bf16) into
    natural-layout natc [s=128p, t, e] tiles; PE-transposed (bf16, 1.0
    cycles/row) and copied PSUM->SBUF as fp8e4 into encT8
    [ki=128p, ko=8, s] (e = ko*128 + ki) for the whole batch up front.
  - W_e is pre-scaled by 2^13 and cast to fp8e4 at setup (the scale lifts
    the uniform(+-0.022) weights out of fp8's subnormal range); the 2^-13
    is folded back in via the tanh's input scale on ACT.
  - Main matmul runs in DoubleRow perf mode: stationary [128,2,128] fp8
    pairs of W_e e-blocks, moving [128,2,512] fp8 pairs of encT8 -> a
    256-deep contraction at 0.5 cycles/row.  Loop order (i, k, c) keeps
    one stationary tile across the 4 chunk matmuls so LDWEIGHTS (256
    cols) amortizes and hides under the 4 matmuls.
  - tanh on ACT with bias=h_proj+attn_b (per-partition scalar in the
    energyT layout) and scale=2^-13; output fp32r.
  - attention = v.T @ energyT on PE in fp32r (fp8 here would cost ~1e-2
    extra rel err - measured too close to the 2e-2 gate).
  - Four chunks' attention logits accumulate in one PSUM bank at
    partition offsets 32c.  exp on ACT (scale folds nothing; v unscaled)
    with accum_out collecting the softmax denominator.  Max-subtraction
    skipped: |logits| <= sum|v| ~ 16, safe in fp32.
  - exp rows transposed back to [s=128p] columns on PE; context
    accumulates on PE in bf16 against the natural-layout enc tiles.
  - The softmax/context chain for batch b is deferred into batch b+1's
    transpose phase so the PE never head-blocks on the ACT exp chain.
"""

import numpy as np

HIDDEN = 1024
ENC = 1024
BATCH = 32
SEQ = 2048
NCORES = 8
B_LOC = BATCH // NCORES  # 4

S_CHUNK = 512
N_CHUNK = SEQ // S_CHUNK  # 4
SUBS = S_CHUNK // 128  # 4
ET = ENC // 128  # 8 e-tiles
HT = HIDDEN // 128  # 8 h-tiles
NPAIR = ET // 2  # 4 DoubleRow e-block pairs

W_SCALE = 8192.0  # 2^13

_CACHED_NC = None


def build_bass(b_loc=B_LOC, seq=SEQ, repeat=1, ablate=(), tune=None):
    ablate = set(ablate)
    T = {
        "nat_bufs": 9,
        "encT_bufs": 6,
        "energyT_bufs": 10,
        "big_bufs": 4,
        "attn_bufs": 2,
        "small_bufs": 24,
        "bf16_nat": True,
    }
    T.update(tune or {})
    import concourse.mybir as mybir
    import concourse.tile as tile
    from concourse import bacc
    from concourse.bass import ts
    from concourse.masks import make_identity

    n_chunk = seq // S_CHUNK

    nc = bacc.Bacc()
    R = mybir.dt.float32r
    F = mybir.dt.float32
    BF = mybir.dt.bfloat16
    F8 = mybir.dt.float8e4
    AF = mybir.ActivationFunctionType
    DR = mybir.MatmulPerfMode.DoubleRow
    NAT = BF if T["bf16_nat"] else R

    hidden = nc.dram_tensor("hidden", [b_loc, HIDDEN], R, kind="ExternalInput")
    enc = nc.dram_tensor("enc", [b_loc, seq, ENC], R, kind="ExternalInput")
    attn_w = nc.dram_tensor("attn_w", [HIDDEN + ENC, HIDDEN], R, kind="ExternalInput")
    attn_b = nc.dram_tensor("attn_b", [HIDDEN], F, kind="ExternalInput")
    v_w = nc.dram_tensor("v_w", [HIDDEN], R, kind="ExternalInput")
    out = nc.dram_tensor("out", [b_loc, ENC], F, kind="ExternalOutput")
    scratch_rz = nc.dram_tensor("scratch_rz", [b_loc, 1], F)

    with tile.TileContext(nc) as tc:
        with (
            tc.tile_pool(name="weights", bufs=1) as w_pool,
            tc.tile_pool(name="consts", bufs=1) as const_pool,
            tc.tile_pool(name="stage", bufs=2) as stage_pool,
            tc.tile_pool(name="nat", bufs=T["nat_bufs"]) as nat_pool,
            tc.tile_pool(name="encT8", bufs=T["encT_bufs"]) as encT8_pool,
            tc.tile_pool(name="energyT", bufs=T["energyT_bufs"]) as energyT_pool,
            tc.tile_pool(name="small", bufs=T["small_bufs"]) as small_pool,
            tc.tile_pool(name="mid", bufs=3) as mid_pool,
            tc.tile_pool(name="ps_big", bufs=T["big_bufs"], space="PSUM") as big_pool,
            tc.tile_pool(
                name="ps_attn", bufs=T["attn_bufs"], space="PSUM"
            ) as attn_pool,
            tc.tile_pool(name="ps_ctx", bufs=1, space="PSUM") as ctx_pool,
        ):
            # ---- constants / weights ----
            ident_f = const_pool.tile([128, 128], F, tag="ident_f")
            make_identity(nc, ident_f[:])
            ident = const_pool.tile([128, 128], R)
            nc.vector.tensor_copy(ident[:], ident_f[:])
            ident_nat = const_pool.tile([128, 128], NAT, tag="ident_nat")
            nc.vector.tensor_copy(ident_nat[:], ident_f[:])

            w_h = w_pool.tile([128, ET, HIDDEN], R, tag="w_h")
            nc.sync.dma_start(
                w_h[:], attn_w[0:HIDDEN, :].rearrange("(j p) h -> p j h", p=128)
            )
            # w8[ki, ko, h] = W_e[ko*128+ki, h] * 2^13 in fp8e4
            w8 = w_pool.tile([128, ET, HIDDEN], F8, tag="w8")
            for j in range(ET):
                wst = stage_pool.tile([128, HIDDEN], R, tag="w_stage")
                nc.sync.dma_start(
                    wst[:], attn_w[HIDDEN + j * 128 : HIDDEN + (j + 1) * 128, :]
                )
                nc.vector.tensor_scalar_mul(w8[:, j, :], wst[:], W_SCALE)

            # v as [h=128p, j] columns, duplicated into 2 cols for the
            # rank-2 attention matmul output rows
            vT2 = const_pool.tile([128, HT, 2], R, tag="vT2")
            for k in range(2):
                nc.gpsimd.dma_start(
                    out=vT2[:, :, k], in_=v_w[:].rearrange("(j p) -> p j", p=128)
                )

            # ---- preamble: hb[b, h] = hidden[b] @ W_h + attn_b, laid out as
            # hbT [h=128p, j, b] per-partition bias columns ----
            hidden_nat = const_pool.tile([b_loc, HIDDEN], R, tag="hidden_nat")
            nc.sync.dma_start(hidden_nat[:], hidden[:, :])

            hiddenT = const_pool.tile([128, ET, b_loc], R, tag="hiddenT")
            for j in range(ET):
                pt = big_pool.tile([128, 512], R, tag="ps")
                nc.tensor.matmul(
                    pt[:, 0:b_loc],
                    hidden_nat[0:b_loc, ts(j, 128)],
                    ident[0:b_loc, 0:b_loc],
                    is_transpose=True,
                    start=True,
                    stop=True,
                )
                nc.vector.tensor_copy(hiddenT[:, j, :], pt[:, 0:b_loc])

            attnb_sb = const_pool.tile([b_loc, HIDDEN], F, tag="attnb")
            nc.gpsimd.dma_start(
                out=attnb_sb[:], in_=attn_b[:].partition_broadcast(b_loc)
            )
            hb_ps = ctx_pool.tile([b_loc, HIDDEN], F, tag="ctx")
            for n in range(2):
                for j in range(ET):
                    nc.tensor.matmul(
                        hb_ps[:, ts(n, 512)],
                        hiddenT[:, j, :],
                        w_h[:, j, ts(n, 512)],
                        start=(j == 0),
                        stop=(j == ET - 1),
                    )
            hb_nat = const_pool.tile([b_loc, HIDDEN], R, tag="hb_nat")
            nc.vector.tensor_add(hb_nat[:], hb_ps[:], attnb_sb[:])

            hbT = const_pool.tile([128, HT, b_loc], R, tag="hbT")
            for j in range(HT):
                pt = big_pool.tile([128, 512], R, tag="ps")
                nc.tensor.matmul(
                    pt[:, 0:b_loc],
                    hb_nat[0:b_loc, ts(j, 128)],
                    ident[0:b_loc, 0:b_loc],
                    is_transpose=True,
                    start=True,
                    stop=True,
                )
                nc.vector.tensor_copy(hbT[:, j, :], pt[:, 0:b_loc])

            # ---- deferred softmax/context emitters (stage A / stage B) ----
            def emit_exp(c, zparts, attn_ps):
                # exp + transposes back to [s=128p] columns; emitted inline
                # right after the chunk pair's attention completes, so ACT
                # runs it while the PE streams the next pair's main matmuls
                if "ctx" in ablate or "attn" in ablate:
                    return []
                exp_row = mid_pool.tile([2, S_CHUNK], R, tag="exp_row")
                nc.scalar.activation(
                    exp_row[:],
                    attn_ps[:],
                    AF.Exp,
                    accum_out=zparts[0:2, c : c + 1],
                )
                pts = []
                for t in range(SUBS):
                    pt = big_pool.tile([128, 2], R, tag="ps")
                    nc.tensor.matmul(
                        pt[:],
                        exp_row[0:2, ts(t, 128)],
                        ident[0:2, 0:2],
                        is_transpose=True,
                        start=True,
                        stop=True,
                    )
                    pts.append(pt)
                ecs = []
                for t in range(SUBS):
                    ec = small_pool.tile([128, 2], NAT, tag="ec")
                    nc.vector.tensor_copy(ec[:], pts[t][:])
                    ecs.append(ec)
                return ecs

            def emit_ctx_mms(c, ctx_ps, ecs, natc):
                # stage B: context accumulation against the natural enc tiles
                if "ctx" in ablate or "attn" in ablate:
                    return
                for t in range(SUBS):
                    for n in range(2):
                        nc.tensor.matmul(
                            ctx_ps[:, ts(n, 512)],
                            ecs[t][:, 0:1],
                            natc[:, t, ts(n, 512)],
                            start=(c == 0 and t == 0),
                            stop=(c == n_chunk - 1 and t == SUBS - 1),
                        )

            def emit_finalize(b, ctx_ps, zparts):
                if "ctx" in ablate or "attn" in ablate:
                    return
                zsum = small_pool.tile([1, 1], F, tag="zsum")
                nc.vector.tensor_reduce(
                    zsum[:],
                    zparts[0:1, :],
                    mybir.AxisListType.X,
                    mybir.AluOpType.add,
                )
                rz = small_pool.tile([1, 1], F, tag="rz")
                nc.vector.reciprocal(rz[:], zsum[:])
                ctx_sb = mid_pool.tile([1, ENC], F, tag="ctx_sb")
                nc.vector.tensor_scalar_mul(ctx_sb[:], ctx_ps[:], rz[:])
                nc.sync.dma_start(out[b : b + 1, :], ctx_sb[:])

            # ---- main loop ----
            def emit_main():
                pending = []  # deferred closures from the previous batch
                for b in range(b_loc):
                    ctx_ps = ctx_pool.tile([1, ENC], F, tag="ctx", name="ctx_ps")
                    zparts = small_pool.tile(
                        [2, n_chunk], F, tag="zparts", name="zparts"
                    )
                    # ---- P1: DMA + transpose the whole batch into encT8 ----
                    natss = []
                    encT8s = []
                    for c in range(n_chunk):
                        # one cast-DMA per chunk: natc[p, t, e] = enc[b,
                        # c*512 + t*128 + p, e], downcast fp32->bf16 inline
                        # (SWDGE).  bf16 halves SBUF + feeds 1.0-cycle/row
                        # PE transposes.
                        natc = nat_pool.tile(
                            [128, SUBS, ENC], NAT, tag="nat", name="natc"
                        )
                        src = enc[b, c * S_CHUNK : (c + 1) * S_CHUNK, :].rearrange(
                            "(t p) e -> p t e", p=128
                        )
                        if T["bf16_nat"]:
                            nc.gpsimd.dma_start(out=natc[:], in_=src)
                        else:
                            nc.sync.dma_start(natc[:], src)
                        natss.append(natc)
                        e8 = encT8_pool.tile(
                            [128, ET, S_CHUNK], F8, tag="encT8", name="e8"
                        )
                        for j in range(ET):
                            if "transpose" not in ablate:
                                pt = big_pool.tile([128, 512], NAT, tag="ps", name="pt")
                                for t in range(SUBS):
                                    nc.tensor.matmul(
                                        pt[:, ts(t, 128)],
                                        natc[:, t, ts(j, 128)],
                                        ident_nat[:],
                                        is_transpose=True,
                                        start=(t == 0),
                                        stop=(t == SUBS - 1),
                                    )
                                nc.vector.tensor_copy(e8[:, j, :], pt[:])
                            # drain deferred ctx/finalize work from the
                            # previous batch between transpose groups so the
                            # PE always has queued work
                            if pending:
                                pending.pop(0)()
                        encT8s.append(e8)
                    while pending:
                        pending.pop(0)()
                    # ---- P2: main matmul + tanh + attention, by chunk pair.
                    # Pairs keep PSUM pressure at 2 attention banks + 2-4 pm
                    # banks, while one LDWEIGHTS still covers 2 matmuls. ----
                    all_ecs = {}
                    for cp in range(n_chunk // 2):
                        pair = (2 * cp, 2 * cp + 1)
                        attns = {}
                        for c in pair:
                            attns[c] = attn_pool.tile(
                                [2, S_CHUNK], F, tag="attn", name="attn_ps"
                            )
                        ets = [[None] * n_chunk for _ in range(HT)]
                        for i in range(HT):
                            pms = {}
                            for c in pair:
                                pms[c] = big_pool.tile(
                                    [128, S_CHUNK], F, tag="ps", name="pm"
                                )
                            if "main" not in ablate:
                                for k in range(NPAIR):
                                    for c in pair:
                                        nc.tensor.matmul(
                                            pms[c][:],
                                            w8[:, 2 * k : 2 * k + 2, ts(i, 128)],
                                            encT8s[c][:, 2 * k : 2 * k + 2, :],
                                            start=(k == 0),
                                            stop=(k == NPAIR - 1),
                                            perf_mode=DR,
                                        )
                            for c in pair:
                                et = energyT_pool.tile(
                                    [128, S_CHUNK], R, tag="energyT", name="et"
                                )
                                if "tanh" not in ablate and "main" not in ablate:
                                    nc.scalar.activation(
                                        et[:],
                                        pms[c][:],
                                        AF.Tanh,
                                        bias=hbT[:, i, b : b + 1],
                                        scale=1.0 / W_SCALE,
                                    )
                                ets[i][c] = et
                            # attention matmul for h-tile i-1: its tanh has
                            # had a full h-tile of main matmuls to complete.
                            if "attn" not in ablate and i > 0:
                                for c in pair:
                                    nc.tensor.matmul(
                                        attns[c][:],
                                        vT2[:, i - 1, :],
                                        ets[i - 1][c][:],
                                        start=(i - 1 == 0),
                                        stop=False,
                                    )
                        if "attn" not in ablate:
                            for c in pair:
                                nc.tensor.matmul(
                                    attns[c][:],
                                    vT2[:, HT - 1, :],
                                    ets[HT - 1][c][:],
                                    start=False,
                                    stop=True,
                                )
                        # exp inline: ACT runs it under the next pair's (or
                        # next batch's) PE work; frees the attn banks early
                        for c in pair:
                            all_ecs[c] = emit_exp(c, zparts, attns[c])
                    # ---- P3 (ctx + finalize, deferred into b+1's P1) ----
                    for c in range(n_chunk):
                        pending.append(
                            lambda c=c, ctx_ps=ctx_ps, natc=natss[c],
                            ecs=all_ecs[c]: emit_ctx_mms(c, ctx_ps, ecs, natc)
                        )
                    pending.append(
                        lambda b=b, ctx_ps=ctx_ps, zparts=zparts: emit_finalize(
                            b, ctx_ps, zparts
                        )
                    )
                for fn in pending:
                    fn()
                del pending[:]

            unroll = T.get("unroll", 1)
            if repeat > 1:
                assert repeat % max(unroll, 1) == 0
                with tc.For_i(0, repeat // max(unroll, 1), 1):
                    for _ in range(max(unroll, 1)):
                        emit_main()
            else:
                emit_main()

    nc.compile()
    return nc


def kernel_run(hidden, encoder_outputs, attn_w, attn_b, v_w, **spmd_kwargs):
    """Shards over batch across 8 cores, runs the Bass kernel SPMD, gathers
    per-core outputs. Returns (full_output, BassKernelResults)."""
    global _CACHED_NC
    from concourse.bass_utils import run_bass_kernel_spmd

    if _CACHED_NC is None:
        _CACHED_NC = build_bass()
    nc = _CACHED_NC

    hidden = np.asarray(hidden, dtype=np.float32).reshape(BATCH, HIDDEN)
    enc = np.ascontiguousarray(np.asarray(encoder_outputs, dtype=np.float32))
    attn_w = np.ascontiguousarray(np.asarray(attn_w, dtype=np.float32))
    attn_b = np.ascontiguousarray(np.asarray(attn_b, dtype=np.float32))
    v_w = np.ascontiguousarray(np.asarray(v_w, dtype=np.float32))

    in_maps = []
    for c in range(NCORES):
        lo, hi = c * B_LOC, (c + 1) * B_LOC
        in_maps.append(
            {
                "hidden": np.ascontiguousarray(hidden[lo:hi]),
                "enc": np.ascontiguousarray(enc[lo:hi]),
                "attn_w": attn_w,
                "attn_b": attn_b,
                "v_w": v_w,
            }
        )

    res = run_bass_kernel_spmd(
        nc, in_maps, core_ids=list(range(NCORES)), **spmd_kwargs
    )
    outs = [r["out"] for r in res.results]
    full = np.concatenate(outs, axis=0).reshape(BATCH, 1, ENC)
    return full, res


def kernel(hidden, encoder_outputs, attn_w, attn_b, v_w):
    """Full-input entry point: takes the full (unsharded) inputs, returns the
    full [32, 1, 1024] output."""
    full, _ = kernel_run(hidden, encoder_outputs, attn_w, attn_b, v_w)
    return full
